# revision 1
# baseline (speedup 1.0000x reference)
"""Trainium2 Bass kernel for nn_Co_Attention (B=256, Nu=Ni=512, D=64).

Math:  S_b = u_fea[b] @ K2 @ i_fea[b].T  with K2 = Wu.T @ M @ Wi  (biases zero)
       p_u = softmax(S.max(axis=2), axis=1);  p_i = softmax(S.max(axis=1), axis=1)

Sharding: data-parallel over batch, 32 batches per core on 8 cores.

Device-side layout (per batch):
  Host pre-transposes u_fea[b] -> uT [64,512] and packs it as UP [128,256]:
    UP[p<64,  c] = uT[p,    c]        (nu in [0,256))
    UP[p>=64, c] = uT[p-64, 256+c]    (nu in [256,512))
  G2x [128,512] = projected i (K2 @ iT), duplicated in both partition halves
  (zero-padded weight matrices let the packed IP layout be the matmul rhs).
  S nu-tile t (=2h+q):  matmul(lhsT=UP[64h:64h+64, 128q:..], rhs=G2x[64h.., :])
  S^T ni-tile t reuses G as the stationary operand (no H projection needed):
    ST_t[:, nu-half h] = matmul(lhsT=G2x[64h.., 128t:128t+128], rhs=UP[64h.., :])
  Row maxes land in score tiles USC/ISC [128, 4*BPC] (col = t*BPC + b); the
  tail transposes them, rearranges to [BPC, 512] and does the softmax.
  Reductions are split between DVE (fused tensor_tensor_reduce max) and ACT
  (evacuating the right half of a tile to SBUF) for SPLIT_N of the 8 units.
"""

import os
import numpy as np

B, NU, NI, D = 256, 512, 512, 64
NCORES = 8
BPC = B // NCORES  # 32

MM_DTYPE = os.environ.get("CO_ATTN_MM_DTYPE", "float32")
SPLIT_N = int(os.environ.get("CO_ATTN_SPLIT_N", "0"))
ST_FROM_G = os.environ.get("CO_ATTN_ST_FROM_G", "1") == "1"

_BUILD_CACHE = {}
last_run_info = {}


def _np_fallback(u_fea, i_fea, M, Wu, bu, Wi, bi):
    u = u_fea.astype(np.float64) @ Wu.T.astype(np.float64) + bu
    i = i_fea.astype(np.float64) @ Wi.T.astype(np.float64) + bi
    S = np.einsum("bue,ef,bif->bui", u, M.astype(np.float64), i)
    us = S.max(axis=2)
    isc = S.max(axis=1)
    pu = np.exp(us - us.max(axis=1, keepdims=True))
    pu /= pu.sum(axis=1, keepdims=True)
    pi = np.exp(isc - isc.max(axis=1, keepdims=True))
    pi /= pi.sum(axis=1, keepdims=True)
    return pu.astype(np.float32)[:, :, None], pi.astype(np.float32)[:, :, None]


def _build_kernel(bpc, mm_dtype, split_n, st_from_g=True):
    """Build + compile the per-core Bass module (same program on all cores)."""
    import concourse.bass as bass
    import concourse.tile as tile
    from concourse import bacc, mybir

    f32 = mybir.dt.float32
    dt_mm = getattr(mybir.dt, mm_dtype)
    X = mybir.AxisListType.X
    MAX = mybir.AluOpType.max
    Exp = mybir.ActivationFunctionType.Exp

    nc = bacc.Bacc("TRN2", debug=False, enable_asserts=True,
                   target_bir_lowering=False)

    ut_d = nc.dram_tensor("ut", [bpc, 128, 256], dt_mm, kind="ExternalInput")
    it_d = nc.dram_tensor("it", [bpc, 128, 256], dt_mm, kind="ExternalInput")
    gwa_d = nc.dram_tensor("gwa", [128, 128], dt_mm, kind="ExternalInput")
    gwb_d = nc.dram_tensor("gwb", [128, 128], dt_mm, kind="ExternalInput")
    hwa_d = nc.dram_tensor("hwa", [128, 128], dt_mm, kind="ExternalInput")
    hwb_d = nc.dram_tensor("hwb", [128, 128], dt_mm, kind="ExternalInput")
    ident_d = nc.dram_tensor("ident", [128, 128], f32, kind="ExternalInput")
    pu_d = nc.dram_tensor("pu", [bpc, 512], f32, kind="ExternalOutput")
    pi_d = nc.dram_tensor("pi", [bpc, 512], f32, kind="ExternalOutput")

    scw = 4 * bpc  # score-tile width

    with tile.TileContext(nc) as tc:
        with (
            tc.tile_pool(name="consts", bufs=1) as cpool,
            tc.tile_pool(name="inp", bufs=6) as ipool,
            tc.tile_pool(name="ghsb", bufs=5) as ghpool,
            tc.tile_pool(name="score", bufs=1) as scpool,
            tc.tile_pool(name="evac", bufs=4) as evpool,
            tc.tile_pool(name="pgh", bufs=2, space="PSUM") as pghpool,
            tc.tile_pool(name="ps", bufs=3, space="PSUM") as pspool,
            tc.tile_pool(name="tail", bufs=2) as tailpool,
        ):
            gwa = cpool.tile([128, 128], dt_mm, tag="gwa")
            gwb = cpool.tile([128, 128], dt_mm, tag="gwb")
            ident = cpool.tile([128, 128], f32, tag="ident")
            nc.sync.dma_start(gwa[:], gwa_d.ap())
            nc.sync.dma_start(gwb[:], gwb_d.ap())
            nc.sync.dma_start(ident[:], ident_d.ap())
            if not st_from_g:
                hwa = cpool.tile([128, 128], dt_mm, tag="hwa")
                hwb = cpool.tile([128, 128], dt_mm, tag="hwb")
                nc.sync.dma_start(hwa[:], hwa_d.ap())
                nc.sync.dma_start(hwb[:], hwb_d.ap())

            USC = scpool.tile([128, scw], f32, tag="usc")
            ISC = scpool.tile([128, scw], f32, tag="isc")
            ISCB = None
            if st_from_g:
                ISCB = scpool.tile([128, scw], f32, tag="iscb")
                nc.gpsimd.memset(ISCB[:], -3.0e38)

            for b in range(bpc):
                up = ipool.tile([128, 256], dt_mm, tag="up")
                nc.sync.dma_start(up[:], ut_d.ap()[b])
                ip = ipool.tile([128, 256], dt_mm, tag="ip")
                nc.scalar.dma_start(ip[:], it_d.ap()[b])

                # G2x = K2 @ iT duplicated into both partition halves.
                g2x_ps = pghpool.tile([128, 512], f32, tag="pgh")
                nc.tensor.matmul(g2x_ps[:, 0:256], gwa[:], ip[:],
                                 start=True, stop=False)
                nc.tensor.matmul(g2x_ps[:, 256:512], gwb[:], ip[:],
                                 start=False, stop=True)
                g2x = ghpool.tile([128, 512], dt_mm, tag="g")
                nc.scalar.copy(g2x[:], g2x_ps[:])

                # S tiles (u-dir): nu-tile t=2h+q in PSUM pair tiles.
                slo = pspool.tile([128, 1024], f32, tag="s")   # t=0,1
                shi = pspool.tile([128, 1024], f32, tag="s")   # t=2,3
                nc.tensor.matmul(slo[:, 0:512], up[0:64, 0:128],
                                 g2x[0:64, :], start=True, stop=True)
                nc.tensor.matmul(shi[:, 0:512], up[64:128, 0:128],
                                 g2x[64:128, :], start=True, stop=True)
                nc.tensor.matmul(slo[:, 512:1024], up[0:64, 128:256],
                                 g2x[0:64, :], start=True, stop=True)
                nc.tensor.matmul(shi[:, 512:1024], up[64:128, 128:256],
                                 g2x[64:128, :], start=True, stop=True)

                tlo = pspool.tile([128, 1024], f32, tag="s")   # ni-tiles 0,1
                thi = pspool.tile([128, 1024], f32, tag="s")   # ni-tiles 2,3
                if st_from_g:
                    # ST tiles (i-dir) from G directly.  Layout per pair tile
                    # [128, 1024]: bank0 (cols 0:512)  = [tA nu-lo | tB nu-lo]
                    #              bank1 (cols 512:1024)= [tA nu-hi | tB nu-hi]
                    # lo-MMs use PE rows 0-63 / bank0; hi-MMs rows 64-127 /
                    # bank1 — concurrent row-group pairs never share a bank.
                    for pair, tA in ((tlo, 0), (thi, 2)):
                        for j, t in enumerate((tA, tA + 1)):
                            nc.tensor.matmul(
                                pair[:, 256 * j:256 * j + 256],
                                g2x[0:64, 128 * t:128 * t + 128],
                                up[0:64, :], start=(j == 0), stop=(j == 1))
                            nc.tensor.matmul(
                                pair[:, 512 + 256 * j:512 + 256 * j + 256],
                                g2x[64:128, 128 * t:128 * t + 128],
                                up[64:128, :], start=(j == 0), stop=(j == 1))
                else:
                    # v1-style: H2x = K2.T @ uT dup'd; ST tiles contiguous.
                    h2x_ps = pghpool.tile([128, 512], f32, tag="pgh")
                    nc.tensor.matmul(h2x_ps[:, 0:256], hwa[:], up[:],
                                     start=True, stop=False)
                    nc.tensor.matmul(h2x_ps[:, 256:512], hwb[:], up[:],
                                     start=False, stop=True)
                    h2x = ghpool.tile([128, 512], dt_mm, tag="h")
                    nc.scalar.copy(h2x[:], h2x_ps[:])
                    nc.tensor.matmul(tlo[:, 0:512], ip[0:64, 0:128],
                                     h2x[0:64, :], start=True, stop=True)
                    nc.tensor.matmul(thi[:, 0:512], ip[64:128, 0:128],
                                     h2x[64:128, :], start=True, stop=True)
                    nc.tensor.matmul(tlo[:, 512:1024], ip[0:64, 128:256],
                                     h2x[0:64, :], start=True, stop=True)
                    nc.tensor.matmul(thi[:, 512:1024], ip[64:128, 128:256],
                                     h2x[64:128, :], start=True, stop=True)

                # Row maxes -> score tiles (col = t*bpc + b).
                # 8 units; unit layout differs between S pairs (contiguous
                # 512-col tile) and ST pairs (two 256-col chunks, one per bank).
                stk = "st" if st_from_g else "s"
                units = [(USC, slo, 0, "s"), (USC, slo, 1, "s"),
                         (USC, shi, 2, "s"), (USC, shi, 3, "s"),
                         (ISC, tlo, 0, stk), (ISC, tlo, 1, stk),
                         (ISC, thi, 2, stk), (ISC, thi, 3, stk)]
                plain = {}
                for idx, (SC, pair, t, kind) in enumerate(units):
                    j = t % 2
                    col = t * bpc + b
                    if idx < split_n:
                        if kind == "s":
                            in0 = pair[:, 512 * j:512 * j + 256]
                            evsrc = pair[:, 512 * j + 256:512 * j + 512]
                        else:
                            in0 = pair[:, 256 * j:256 * j + 256]
                            evsrc = pair[:, 512 + 256 * j:512 + 256 * j + 256]
                        ev = evpool.tile([128, 256], f32, tag="ev")
                        nc.scalar.copy(ev[:], evsrc)
                        scr = evpool.tile([128, 256], f32, tag="scr")
                        nc.vector.tensor_tensor_reduce(
                            out=scr[:], in0=in0, in1=ev[:],
                            scale=1.0, scalar=-3.0e38,
                            op0=MAX, op1=MAX,
                            accum_out=SC[:, col:col + 1])
                    else:
                        plain.setdefault(id(pair), []).append((SC, pair, t, kind))
                for group in plain.values():
                    SC, pair, t0, kind = group[0]
                    if len(group) == 2:
                        b0 = (t0 - t0 % 2) * bpc + b
                        if kind == "s":
                            nc.vector.reduce_max(
                                SC[:, b0:b0 + bpc + 1:bpc],
                                pair[:].rearrange("p (t n) -> p t n", t=2),
                                axis=X)
                        else:
                            # chunked ST layout: lo-chunk maxes -> ISC,
                            # hi-chunk maxes -> ISCB; combined after loop.
                            nc.vector.reduce_max(
                                SC[:, b0:b0 + bpc + 1:bpc],
                                pair[:, 0:512].rearrange("p (t n) -> p t n", t=2),
                                axis=X)
                            nc.vector.reduce_max(
                                ISCB[:, b0:b0 + bpc + 1:bpc],
                                pair[:, 512:1024].rearrange("p (t n) -> p t n", t=2),
                                axis=X)
                    else:
                        for SC, pair, t, kind in group:
                            j = t % 2
                            col = t * bpc + b
                            if kind == "s":
                                nc.vector.reduce_max(
                                    SC[:, col:col + 1],
                                    pair[:, 512 * j:512 * j + 512], axis=X)
                            else:
                                nc.vector.reduce_max(
                                    SC[:, col:col + 1],
                                    pair[:].rearrange(
                                        "p (c t n) -> p t c n", c=2, t=2)[:, j],
                                    axis=mybir.AxisListType.XY)

            if st_from_g:
                nc.vector.tensor_tensor(ISC[:], ISC[:], ISCB[:], op=MAX)

            # ---- softmax tail (once per core) ----
            for SC, out_d in ((USC, pu_d), (ISC, pi_d)):
                sct_ps = pghpool.tile([scw, 128], f32, tag="pgh")
                nc.tensor.transpose(sct_ps[:], SC[:], ident[:])
                sct = tailpool.tile([scw, 128], f32, tag="sct")
                nc.scalar.copy(sct[:], sct_ps[:])
                v = tailpool.tile([bpc, 512], f32, tag="v")
                for t in range(4):
                    nc.sync.dma_start(v[:, 128 * t:128 * (t + 1)],
                                      sct[bpc * t:bpc * (t + 1), :])
                m = tailpool.tile([bpc, 1], f32, tag="m")
                nc.vector.reduce_max(m[:], v[:], axis=X)
                negm = tailpool.tile([bpc, 1], f32, tag="negm")
                nc.scalar.mul(negm[:], m[:], -1.0)
                e = tailpool.tile([bpc, 512], f32, tag="e")
                esum = tailpool.tile([bpc, 1], f32, tag="esum")
                nc.scalar.activation(e[:], v[:], Exp, bias=negm[:], scale=1.0,
                                     accum_out=esum[:])
                rs = tailpool.tile([bpc, 1], f32, tag="rs")
                nc.vector.reciprocal(rs[:], esum[:])
                p = tailpool.tile([bpc, 512], f32, tag="p")
                nc.vector.tensor_scalar_mul(p[:], e[:], rs[:])
                nc.sync.dma_start(out_d.ap(), p[:])

    nc.compile()
    return nc


def _get_kernel(bpc, mm_dtype, split_n, st_from_g=True):
    key = (bpc, mm_dtype, split_n, st_from_g)
    if key not in _BUILD_CACHE:
        _BUILD_CACHE[key] = _build_kernel(bpc, mm_dtype, split_n, st_from_g)
    return _BUILD_CACHE[key]


def _host_pack(xT):  # [n, 64, 512] -> packed [n, 128, 256]
    n = xT.shape[0]
    return np.ascontiguousarray(
        xT.reshape(n, 64, 2, 256).transpose(0, 2, 1, 3).reshape(n, 128, 256))


def kernel(u_fea, i_fea, M, Wu, bu, Wi, bi):
    u_fea = np.asarray(u_fea, dtype=np.float32)
    i_fea = np.asarray(i_fea, dtype=np.float32)
    M = np.asarray(M, dtype=np.float32)
    Wu = np.asarray(Wu, dtype=np.float32)
    Wi = np.asarray(Wi, dtype=np.float32)
    bu = np.asarray(bu, dtype=np.float32)
    bi = np.asarray(bi, dtype=np.float32)

    if np.any(bu) or np.any(bi):
        # Zero biases are guaranteed by the problem spec; handle the general
        # case on host for safety.
        return _np_fallback(u_fea, i_fea, M, Wu, bu, Wi, bi)

    from concourse.bass_utils import run_bass_kernel_spmd

    K2 = (Wu.T.astype(np.float64) @ M.astype(np.float64)
          @ Wi.astype(np.float64)).astype(np.float32)
    K2T_dup = np.concatenate([K2.T, K2.T], axis=1)        # [64,128]
    Z = np.zeros_like(K2T_dup)
    gwa = np.ascontiguousarray(np.concatenate([K2T_dup, Z], axis=0))
    gwb = np.ascontiguousarray(np.concatenate([Z, K2T_dup], axis=0))
    K2_dup = np.concatenate([K2, K2], axis=1)
    hwa = np.ascontiguousarray(np.concatenate([K2_dup, Z], axis=0))
    hwb = np.ascontiguousarray(np.concatenate([Z, K2_dup], axis=0))
    ident = np.eye(128, dtype=np.float32)

    uT = np.ascontiguousarray(u_fea.transpose(0, 2, 1))   # [B, 64, 512]
    iT = np.ascontiguousarray(i_fea.transpose(0, 2, 1))
    up = _host_pack(uT)                                   # [B, 128, 256]
    ip = _host_pack(iT)

    nc = _get_kernel(BPC, MM_DTYPE, SPLIT_N, ST_FROM_G)

    in_maps = []
    for c in range(NCORES):
        in_maps.append({
            "ut": up[c * BPC:(c + 1) * BPC],
            "it": ip[c * BPC:(c + 1) * BPC],
            "gwa": gwa, "gwb": gwb, "hwa": hwa, "hwb": hwb,
            "ident": ident,
        })

    trace = os.environ.get("CO_ATTN_TRACE", "0") == "1"
    res = run_bass_kernel_spmd(nc, in_maps, core_ids=list(range(NCORES)),
                               trace=trace)
    last_run_info.clear()
    last_run_info.update({
        "exec_time_ns": res.exec_time_ns,
        "mean_exec_time_ns": res.mean_exec_time_ns,
        "results_obj": res,
    })

    p_u = np.concatenate([res.results[c]["pu"] for c in range(NCORES)], axis=0)
    p_i = np.concatenate([res.results[c]["pi"] for c in range(NCORES)], axis=0)
    return p_u[:, :, None].astype(np.float32), p_i[:, :, None].astype(np.float32)



# revision 2
# speedup vs baseline: 1.0318x; 1.0318x over previous
"""Trainium2 Bass kernel v4.1 for nn_Co_Attention (B=256, Nu=Ni=512, D=64).

Math:  S_b = u_fea[b] @ K2 @ i_fea[b].T,  K2 = Wu.T @ M @ Wi  (biases zero)
       p_u = softmax(S.max(axis=2), axis=1);  p_i = softmax(S.max(axis=1), axis=1)

Design (constraints discovered by microbenchmarking this hardware):
 - tensor_tensor_reduce crashes at runtime (even the v1 SPLIT_N path);
   ops can read at most ONE PSUM operand; Pool(gpsimd) supports no
   elementwise/reduce ops (only memset / partition_* customs / DMA issue);
   ACT cannot max; DVE reduce = 1 elem/cycle/partition @0.96GHz; compute
   engines cannot address partition bases other than 0/32/64/96.
 - Single S-pass in fp16 (split hi/lo operands: ~6.9e-3 max-rel, gate 2e-2).
   All matmuls use K=64 stationaries placed in both partition halves so
   consecutive instructions run on PE row groups h0/h64 concurrently.
 - ACT evacuates the 4 S tiles PSUM->SBUF (two [128,1024] copies).
 - DVE: ONE strided rowmax over [128,(4),512] -> 4 USC cols per batch,
   then a 2-op tensor_tensor max tree -> T[128,512].
 - Pool: partition_all_reduce(max) over T -> i_score row; DMA copies row 0
   into IV[b] (SBUF->SBUF).
 - Tail: u via PE transpose of USC; i softmaxes IV [32,512] directly.
"""

import os
import numpy as np

B, NU, NI, D = 256, 512, 512, 64
NCORES = 8
BPC = B // NCORES  # 32

_BUILD_CACHE = {}
last_run_info = {}


def _build_kernel_v4(bpc):
    import concourse.bass as bass
    import concourse.tile as tile
    from concourse import bacc, mybir, bass_isa

    f32 = mybir.dt.float32
    f16 = mybir.dt.float16
    X = mybir.AxisListType.X
    MAX = mybir.AluOpType.max
    Exp = mybir.ActivationFunctionType.Exp

    nc = bacc.Bacc("TRN2", debug=False, enable_asserts=True,
                   target_bir_lowering=False)

    up2_d = nc.dram_tensor("up2", [bpc, 128, 512], f16, kind="ExternalInput")
    ih2_d = nc.dram_tensor("ih2", [bpc, 128, 1024], f16, kind="ExternalInput")
    k2h_d = nc.dram_tensor("k2h", [128, 64], f16, kind="ExternalInput")
    k2l_d = nc.dram_tensor("k2l", [128, 64], f16, kind="ExternalInput")
    ident_d = nc.dram_tensor("ident", [128, 128], f32, kind="ExternalInput")
    pu_d = nc.dram_tensor("pu", [bpc, 512], f32, kind="ExternalOutput")
    pi_d = nc.dram_tensor("pi", [bpc, 512], f32, kind="ExternalOutput")

    scw = 4 * bpc  # 128

    with tile.TileContext(nc) as tc:
        with (
            tc.tile_pool(name="consts", bufs=1) as cpool,
            tc.tile_pool(name="inp", bufs=8) as ipool,
            tc.tile_pool(name="gsb", bufs=2) as gpool,
            tc.tile_pool(name="scopy", bufs=2) as scpool_sb,
            tc.tile_pool(name="tt", bufs=2) as ttpool,
            tc.tile_pool(name="pout", bufs=2) as popool,
            tc.tile_pool(name="score", bufs=1) as scpool,
            tc.tile_pool(name="pg", bufs=2, space="PSUM") as pgpool,
            tc.tile_pool(name="pss", bufs=3, space="PSUM") as psspool,
            tc.tile_pool(name="tail", bufs=2) as tailpool,
        ):
            # k2h/k2l: K2.T stacked in both partition halves [128, 64]
            k2h = cpool.tile([128, 64], f16, tag="k2h")
            k2l = cpool.tile([128, 64], f16, tag="k2l")
            ident = cpool.tile([128, 128], f32, tag="ident")
            nc.sync.dma_start(k2h[:], k2h_d.ap())
            nc.sync.dma_start(k2l[:], k2l_d.ap())
            nc.sync.dma_start(ident[:], ident_d.ap())

            USC = scpool.tile([128, scw], f32, tag="usc")
            IV = scpool.tile([bpc, 512], f32, tag="iv")

            def load_inputs(b):
                up2 = ipool.tile([128, 512], f16, tag="up2")
                ih2 = ipool.tile([128, 1024], f16, tag="ih2")
                nc.sync.dma_start(up2[:], up2_d.ap()[b])
                nc.sync.dma_start(ih2[:], ih2_d.ap()[b])
                return up2, ih2

            def proj(ih2):
                # G = K2 @ iT, duplicated in both halves; K=64 stationaries at
                # partition bases 0 / 64 -> h0/h64 run concurrently.
                ihd, ild = ih2[:, 0:512], ih2[:, 512:1024]
                g_ps = pgpool.tile([128, 512], f32, tag="g")
                terms = ((k2h, ihd, True, False), (k2h, ild, False, False),
                         (k2l, ihd, False, True))
                for w, x, st, sp in terms:
                    nc.tensor.matmul(g_ps[0:64, :], w[0:64, :], x[0:64, :],
                                     start=st, stop=sp)
                    nc.tensor.matmul(g_ps[64:128, :], w[64:128, :],
                                     x[64:128, :], start=st, stop=sp)
                g2x = gpool.tile([128, 512], f16, tag="g2x")
                nc.scalar.copy(g2x[:], g_ps[:])
                return g2x

            # prologue: inputs + projection for batch 0 (and input for 1)
            ins = [load_inputs(0)]
            g2xs = [proj(ins[0][1])]

            for b in range(bpc):
                up2, _ = ins[b]
                g2x = g2xs[b]
                uph, upl = up2[:, 0:256], up2[:, 256:512]

                # ---- S-pass: tiles t: pair0=(t0,t1) h0, pair1=(t2,t3) h64.
                pair0 = psspool.tile([128, 1024], f32, tag="s")
                pair1 = psspool.tile([128, 1024], f32, tag="s")
                for up, st, sp in ((uph, True, False), (upl, False, True)):
                    for q in (0, 1):
                        nc.tensor.matmul(pair0[:, 512 * q:512 * q + 512],
                                         up[0:64, 128 * q:128 * q + 128],
                                         g2x[0:64, :], start=st, stop=sp)
                        nc.tensor.matmul(pair1[:, 512 * q:512 * q + 512],
                                         up[64:128, 128 * q:128 * q + 128],
                                         g2x[64:128, :], start=st, stop=sp)

                # lookahead: inputs + projection for batch b+1 queue BEFORE
                # this batch's big ACT evacs so the next S-pass isn't gated.
                if b + 1 < bpc:
                    ins.append(load_inputs(b + 1))
                    g2xs.append(proj(ins[b + 1][1]))

                # ---- ACT evacuates all 4 tiles into one [128,2048] copy
                # sc tile order: t0 | t1 | t2 | t3 (nu chunks 0..3 x 128)
                sc = scpool_sb.tile([128, 2048], f32, tag="sc")
                nc.scalar.copy(sc[:, 0:1024], pair0[:])
                nc.scalar.copy(sc[:, 1024:2048], pair1[:])

                # ---- u_score: ONE strided rowmax over [128,(4),512]
                nc.vector.reduce_max(
                    USC[:, b:b + 3 * bpc + 1:bpc],
                    sc[:].rearrange("p (t n) -> p t n", t=4), axis=X)

                # ---- T = elementwise max over the 4 tiles (2 ops)
                tt = ttpool.tile([128, 1024], f32, tag="tt")
                nc.vector.tensor_tensor(tt[:], sc[:, 0:1024],
                                        sc[:, 1024:2048], op=MAX)
                tf = ttpool.tile([128, 512], f32, tag="tf")
                nc.vector.tensor_tensor(tf[:], tt[:, 0:512],
                                        tt[:, 512:1024], op=MAX)

                # ---- i_score row: partition max on gpsimd; row 0 -> IV[b]
                po = popool.tile([128, 512], f32, tag="po")
                nc.gpsimd.partition_all_reduce(
                    po[:], tf[:], channels=128,
                    reduce_op=bass_isa.ReduceOp.max)
                nc.gpsimd.dma_start(IV[b:b + 1, :], po[0:1, :])

            # ---- softmax tails ----
            # u: transpose USC -> [scw,128] -> v [bpc,512] -> softmax
            sct_ps = pgpool.tile([scw, 128], f32, tag="g")
            nc.tensor.transpose(sct_ps[:], USC[:], ident[:])
            sct = tailpool.tile([scw, 128], f32, tag="sct")
            nc.scalar.copy(sct[:], sct_ps[:])
            v = tailpool.tile([bpc, 512], f32, tag="v")
            for t in range(4):
                nc.sync.dma_start(v[:, 128 * t:128 * (t + 1)],
                                  sct[bpc * t:bpc * (t + 1), :])

            for src, out_d in ((v, pu_d), (IV, pi_d)):
                m = tailpool.tile([bpc, 1], f32, tag="m")
                nc.vector.reduce_max(m[:], src[:], axis=X)
                negm = tailpool.tile([bpc, 1], f32, tag="negm")
                nc.scalar.mul(negm[:], m[:], -1.0)
                e = tailpool.tile([bpc, 512], f32, tag="e")
                esum = tailpool.tile([bpc, 1], f32, tag="esum")
                nc.scalar.activation(e[:], src[:], Exp, bias=negm[:], scale=1.0,
                                     accum_out=esum[:])
                rs = tailpool.tile([bpc, 1], f32, tag="rs")
                nc.vector.reciprocal(rs[:], esum[:])
                pout = tailpool.tile([bpc, 512], f32, tag="p")
                nc.vector.tensor_scalar_mul(pout[:], e[:], rs[:])
                nc.sync.dma_start(out_d.ap(), pout[:])

    nc.compile()
    return nc


def _get_kernel(bpc):
    if bpc not in _BUILD_CACHE:
        _BUILD_CACHE[bpc] = _build_kernel_v4(bpc)
    return _BUILD_CACHE[bpc]


def _host_pack(xT):  # [n, 64, 512] -> packed [n, 128, 256]
    n = xT.shape[0]
    return np.ascontiguousarray(
        xT.reshape(n, 64, 2, 256).transpose(0, 2, 1, 3).reshape(n, 128, 256))


def _split16(x):
    h = x.astype(np.float16)
    l = (x.astype(np.float32) - h.astype(np.float32)).astype(np.float16)
    return h, l


def kernel(u_fea, i_fea, M, Wu, bu, Wi, bi):
    u_fea = np.asarray(u_fea, dtype=np.float32)
    i_fea = np.asarray(i_fea, dtype=np.float32)
    M = np.asarray(M, dtype=np.float32)
    Wu = np.asarray(Wu, dtype=np.float32)
    Wi = np.asarray(Wi, dtype=np.float32)
    bu = np.asarray(bu, dtype=np.float32)
    bi = np.asarray(bi, dtype=np.float32)
    assert not np.any(bu) and not np.any(bi), "nonzero biases unsupported"

    from concourse.bass_utils import run_bass_kernel_spmd

    K2 = (Wu.T.astype(np.float64) @ M.astype(np.float64)
          @ Wi.astype(np.float64)).astype(np.float32)
    K2h, K2l = _split16(K2)
    k2h = np.ascontiguousarray(np.concatenate([K2h.T, K2h.T], axis=0))  # [128,64]
    k2l = np.ascontiguousarray(np.concatenate([K2l.T, K2l.T], axis=0))
    ident = np.eye(128, dtype=np.float32)

    uT = np.ascontiguousarray(u_fea.transpose(0, 2, 1))   # [B, 64, 512]
    iT = np.ascontiguousarray(i_fea.transpose(0, 2, 1))
    up = _host_pack(uT)                                   # [B, 128, 256] f32
    uph, upl = _split16(up)
    up2 = np.ascontiguousarray(np.concatenate([uph, upl], axis=2))  # [B,128,512]
    iTh, iTl = _split16(iT)                               # [B, 64, 512] f16
    ihd = np.concatenate([iTh, iTh], axis=1)              # [B, 128, 512]
    ild = np.concatenate([iTl, iTl], axis=1)
    ih2 = np.ascontiguousarray(np.concatenate([ihd, ild], axis=2))  # [B,128,1024]

    nc = _get_kernel(BPC)

    in_maps = []
    for c in range(NCORES):
        sl = slice(c * BPC, (c + 1) * BPC)
        in_maps.append({
            "up2": up2[sl], "ih2": ih2[sl],
            "k2h": k2h, "k2l": k2l,
            "ident": ident,
        })

    trace = os.environ.get("CO_ATTN_TRACE", "0") == "1"
    res = run_bass_kernel_spmd(nc, in_maps, core_ids=list(range(NCORES)),
                               trace=trace)
    last_run_info.clear()
    last_run_info.update({
        "exec_time_ns": res.exec_time_ns,
        "mean_exec_time_ns": res.mean_exec_time_ns,
        "results_obj": res,
    })

    p_u = np.concatenate([res.results[c]["pu"] for c in range(NCORES)], axis=0)
    p_i = np.concatenate([res.results[c]["pi"] for c in range(NCORES)], axis=0)
    return p_u[:, :, None].astype(np.float32), p_i[:, :, None].astype(np.float32)
